# revision 1
# baseline (speedup 1.0000x reference)
"""DGI (Deep Graph Infomax) forward kernel for 8 TRN2 NeuronCores.

Problem (all shapes hardcoded):
  seq1, seq2: [1, 8192, 128] f32   node features
  adj:        [1, 8192, 8192] f32  dense adjacency
  cc_label:   [8, 1024] i32        community partition (arange layout)
  W: [128,128], b: [128], Wb: [128,128], bb: [] f32
  out:        [1, 16384] f32       = concat(ret1, ret2)

Math per GCN branch: h = relu(adj @ (seq @ W) + b). We reassociate to
(adj @ seq) @ W so the big contraction uses natural-layout seq tiles as
the stationary operand and a host-transposed adj block as the moving
operand; everything then lives in "transposed" space (features on
partitions), where the community mean is a free-axis reduction and the
bilinear scores are a 1-column matmul.

Sharding: core k owns nodes [1024k, 1024k+1024) == community k (cc_label
is arange). Each core reads its adjT column block (32 MB), the full seqs
(8 MB, replicated), computes its 1024 scores per branch. No collectives.

Per-core device program (big matmuls in fp16: adj is pre-scaled by 256 on
the host to sit in fp16's normal range; the scale is undone for free in the
relu activation's `scale`; everything downstream is fp32):
  ZT[d, n]   = sum_m seq_s[m, d] * adjT[m, n]   (fp16, 64 accumulating
                                                 matmuls per psum bank,
                                                 split into two m-halves so
                                                 half 1's epilogue overlaps
                                                 half 2's stream)
  aggT[h, n] = sum_d W[d, h] * ZT[d, n]         (fp32)
  hT         = relu(aggT/256 + b)   (+ free-axis accum -> community sum)
  c          = sigmoid(sum / 1024)               [128, 1]
  cw         = Wb @ c     (lhsT = Wb^T from host) [128, 1]
  sc_s[n]    = sum_h hT_s[h, n] * cw[h] + bb     [1, 1024] per branch

Layouts are host-prepared so every DMA is partition-major with >=2KB
contiguous per-partition runs: adjt[p, t, n] = adj[node n of this core's
block, 128*t + p] * 256 (fp16), seq[p, t, d] = seq[128*t + p, d] (fp16).
adjacency streams on the sync HWDGE queue, seqs on the scalar queue, params
on gpsimd, so none of them serialize behind each other.
"""

import numpy as np

import concourse.bass as bass
import concourse.tile as tile
from concourse import bacc, mybir
from concourse.bass_utils import run_bass_kernel_spmd

N = 8192          # nodes
D = 128           # input feature dim
H = 128           # hidden dim
NC = 8            # communities / cores
CS = N // NC      # community size (nodes per core)
MT = N // 128     # number of 128-row m-tiles (64)
CHUNK = 512       # matmul moving free dim (psum bank width in fp32)
NCH = CS // CHUNK # n-chunks per core (2)

F32 = mybir.dt.float32
F16 = mybir.dt.float16
ADJ_SCALE = 256.0  # keeps fp16(adj*scale) in the normal range; undone in the relu


def _build_module() -> bass.Bass:
    nc = bacc.Bacc()

    adjt = nc.declare_dram_parameter("adjt", [128, MT, CS], F16, isOutput=False)
    seq1 = nc.declare_dram_parameter("seq1", [128, MT, D], F16, isOutput=False)
    seq2 = nc.declare_dram_parameter("seq2", [128, MT, D], F16, isOutput=False)
    w = nc.declare_dram_parameter("w", [D, H], F32, isOutput=False)
    wbt = nc.declare_dram_parameter("wbt", [H, H], F32, isOutput=False)
    bvec = nc.declare_dram_parameter("bvec", [H, 1], F32, isOutput=False)
    bbvec = nc.declare_dram_parameter("bbvec", [1, 1], F32, isOutput=False)
    out = nc.declare_dram_parameter("out", [2, CS], F32, isOutput=True)

    with tile.TileContext(nc) as tc:
        _emit(tc, adjt, seq1, seq2, w, wbt, bvec, bbvec, out)
    nc.finalize()
    return nc


def _emit(tc, adjt, seq1, seq2, w, wbt, bvec, bbvec, out):
    nc = tc.nc
    with (
        tc.tile_pool(name="singles", bufs=1) as singles,
        tc.tile_pool(name="adj_pool", bufs=3) as adj_pool,
        tc.tile_pool(name="misc", bufs=1) as misc,
        tc.tile_pool(name="psum", bufs=1, space="PSUM") as psum,
    ):
        w_sb = singles.tile([D, H], F32)
        nc.gpsimd.dma_start(out=w_sb, in_=w[:])
        wbt_sb = singles.tile([H, H], F32)
        nc.gpsimd.dma_start(out=wbt_sb, in_=wbt[:])
        b_sb = singles.tile([H, 1], F32)
        nc.gpsimd.dma_start(out=b_sb, in_=bvec[:])
        bb_sb = singles.tile([1, 1], F32)
        nc.gpsimd.dma_start(out=bb_sb, in_=bbvec[:])

        # Warmup adjacency group issued first on the scalar queue so the PE
        # can start while sync's first big group is in flight.
        WARM = 2
        adj_warm = adj_pool.tile([128, WARM, CS], F16, name="adj_warm", bufs=1)
        nc.scalar.dma_start(out=adj_warm, in_=adjt[:, 0:WARM, :])

        # Seqs staged in growing chunks so the first matmul isn't gated on
        # the full 4 MB.
        seq1_sb = singles.tile([128, MT, D], F16)
        seq2_sb = singles.tile([128, MT, D], F16)
        SEQ_CHUNKS = [4, 4, 8, 16, 16, 16]
        pos = 0
        for n in SEQ_CHUNKS:
            sl = slice(pos, pos + n)
            nc.scalar.dma_start(out=seq1_sb[:, sl, :], in_=seq1[:, sl, :])
            nc.scalar.dma_start(out=seq2_sb[:, sl, :], in_=seq2[:, sl, :])
            pos += n
        seq_sb = (seq1_sb, seq2_sb)

        HALF = MT // 2
        # Z accumulators split by m-half: first half banks 0-3, second 4-7.
        z_half = [
            [
                [psum.tile([128, CHUNK], F32, name=f"z_ps_{h}_{s}_{c}") for c in range(NCH)]
                for s in range(2)
            ]
            for h in range(2)
        ]
        zt_sb = [
            [
                [misc.tile([128, CHUNK], F32, name=f"zt_sb_{h}_{s}_{c}") for c in range(NCH)]
                for s in range(2)
            ]
            for h in range(2)
        ]
        h_sb = [
            [misc.tile([128, CHUNK], F32, name=f"h_sb_{s}_{c}") for c in range(NCH)]
            for s in range(2)
        ]
        csum = [misc.tile([H, 1], F32, name=f"csum_{c}") for c in range(NCH)]

        # (queue, n_tiles): tiny warmup group on the scalar queue lets the
        # PE start while sync's first big group is still in flight.
        ADJ_GROUPS = [("w", WARM), ("s", 6)] + [("s", 8)] * 7
        assert sum(n for _, n in ADJ_GROUPS) == MT

        def copy_ps(dst, src_ps, c):
            if c == 0:
                nc.vector.tensor_copy(out=dst, in_=src_ps)
            else:
                nc.scalar.activation(
                    out=dst, in_=src_ps, func=mybir.ActivationFunctionType.Copy
                )

        def emit_half1_copies():
            for s in range(2):
                for c in range(NCH):
                    copy_ps(zt_sb[0][s][c], z_half[0][s][c], c)

        def emit_half1_agg():
            # First-pass W-contraction into the (now free) first-half banks.
            for s in range(2):
                for c in range(NCH):
                    nc.tensor.matmul(
                        z_half[0][s][c], w_sb, zt_sb[0][s][c], start=True, stop=False
                    )

        t0 = 0
        for gi, (q, gn) in enumerate(ADJ_GROUPS):
            if q == "w":
                adj_sb = adj_warm
            else:
                adj_sb = adj_pool.tile([128, gn, CS], F16, name="adj_sb", tag="adj_sb", bufs=4)
                nc.sync.dma_start(out=adj_sb, in_=adjt[:, t0 : t0 + gn, :])
            for u in range(gn):
                t = t0 + u
                h = 0 if t < HALF else 1
                for s in range(2):
                    lhsT = seq_sb[s][:, t, :]
                    for c in range(NCH):
                        nc.tensor.matmul(
                            z_half[h][s][c],
                            lhsT,
                            adj_sb[:, u, c * CHUNK : (c + 1) * CHUNK],
                            start=(t % HALF == 0),
                            stop=(t % HALF == HALF - 1),
                        )
            t0 += gn
            if t0 - gn < HALF <= t0:
                emit_half1_copies()
            if t0 - gn < HALF + 16 <= t0:
                emit_half1_agg()

        # Tail: branch 0 (drives the sigmoid/cw chain) first; branch 1's
        # matmuls/relu fill the PE while scalar runs sigmoid.
        for c in range(NCH):
            copy_ps(zt_sb[1][0][c], z_half[1][0][c], c)
        for c in range(NCH):
            nc.tensor.matmul(
                z_half[0][0][c], w_sb, zt_sb[1][0][c], start=False, stop=True
            )
            nc.scalar.activation(
                out=h_sb[0][c],
                in_=z_half[0][0][c],
                func=mybir.ActivationFunctionType.Relu,
                bias=b_sb,
                scale=1.0 / ADJ_SCALE,
                accum_out=csum[c],
            )
        for c in range(NCH):
            copy_ps(zt_sb[1][1][c], z_half[1][1][c], c)

        csum_tot = misc.tile([H, 1], F32)
        nc.vector.tensor_add(out=csum_tot, in0=csum[0], in1=csum[1])
        c_sb = misc.tile([H, 1], F32)
        nc.scalar.activation(
            out=c_sb,
            in_=csum_tot,
            func=mybir.ActivationFunctionType.Sigmoid,
            scale=1.0 / CS,
        )

        for c in range(NCH):
            nc.tensor.matmul(
                z_half[0][1][c], w_sb, zt_sb[1][1][c], start=False, stop=True
            )
        cw_ps = z_half[1][0][0]
        nc.tensor.matmul(cw_ps[:, :1], wbt_sb, c_sb, start=True, stop=True)
        for c in range(NCH):
            nc.scalar.activation(
                out=h_sb[1][c],
                in_=z_half[0][1][c],
                func=mybir.ActivationFunctionType.Relu,
                bias=b_sb,
                scale=1.0 / ADJ_SCALE,
            )
        cw_sb = misc.tile([H, 1], F32)
        nc.vector.tensor_copy(out=cw_sb, in_=cw_ps[:, :1])

        out_sb = misc.tile([1, 2, CS], F32)
        sc_banks = [
            [z_half[1][0][1], z_half[1][1][0]],
            [z_half[1][1][1], z_half[0][0][0]],
        ]
        for s in range(2):
            for c in range(NCH):
                nc.tensor.matmul(
                    sc_banks[s][c][:1, :], cw_sb, h_sb[s][c], start=True, stop=True
                )
            for c in range(NCH):
                nc.vector.tensor_scalar_add(
                    out=out_sb[:, s, c * CHUNK : (c + 1) * CHUNK],
                    in0=sc_banks[s][c][:1, :],
                    scalar1=bb_sb,
                )
            nc.gpsimd.dma_start(
                out=out[s : s + 1, :].unsqueeze(0), in_=out_sb[:, s, :].unsqueeze(1)
            )


_MODULE_CACHE: list = []


def get_module() -> bass.Bass:
    if not _MODULE_CACHE:
        _MODULE_CACHE.append(_build_module())
    return _MODULE_CACHE[0]


def shard_inputs(inputs: dict) -> list[dict]:
    """Full inputs -> per-core input maps (row-block sharding of adjT)."""
    def tile_seq(s):
        s16 = np.asarray(s, np.float32)[0].astype(np.float16)  # [N, D]
        return np.ascontiguousarray(s16.reshape(MT, 128, D).transpose(1, 0, 2))

    seq1 = tile_seq(inputs["seq1"])
    seq2 = tile_seq(inputs["seq2"])
    adj16 = (np.asarray(inputs["adj"], np.float32)[0] * ADJ_SCALE).astype(np.float16)
    w = np.ascontiguousarray(np.asarray(inputs["W"], np.float32))
    wbt = np.ascontiguousarray(np.asarray(inputs["Wb"], np.float32).T)
    bvec = np.asarray(inputs["b"], np.float32).reshape(H, 1).copy()
    bbvec = np.asarray(inputs["bb"], np.float32).reshape(1, 1).copy()

    in_maps = []
    for k in range(NC):
        in_maps.append(
            {
                "adjt": np.ascontiguousarray(
                    adj16[k * CS : (k + 1) * CS, :].T.reshape(MT, 128, CS).transpose(1, 0, 2)
                ),
                "seq1": seq1,
                "seq2": seq2,
                "w": w,
                "wbt": wbt,
                "bvec": bvec,
                "bbvec": bbvec,
            }
        )
    return in_maps


def gather_output(core_outs: list[np.ndarray], cc_label: np.ndarray) -> np.ndarray:
    """Per-core [2, CS] score blocks -> full [1, 2N] output.

    Scatter through cc_label mirrors the reference's .at[flat].set: entry
    (community k, position j) is the score of node cc_label[k, j].
    """
    sc1 = np.concatenate([o[0] for o in core_outs]).astype(np.float32)
    sc2 = np.concatenate([o[1] for o in core_outs]).astype(np.float32)
    flat = np.asarray(cc_label).reshape(-1)
    ret1 = np.zeros(N, np.float32)
    ret2 = np.zeros(N, np.float32)
    ret1[flat] = sc1
    ret2[flat] = sc2
    return np.concatenate([ret1, ret2])[None, :]


def kernel(**inputs) -> np.ndarray:
    nc = get_module()
    in_maps = shard_inputs(inputs)
    res = run_bass_kernel_spmd(nc, in_maps, core_ids=list(range(NC)))
    core_outs = [res.results[k]["out"] for k in range(NC)]
    return gather_output(core_outs, inputs["cc_label"])


if __name__ == "__main__":
    nc = get_module()
    print("module built ok")



# revision 7
# speedup vs baseline: 1.0279x; 1.0279x over previous
"""DGI (Deep Graph Infomax) forward kernel for 8 TRN2 NeuronCores.

Problem (all shapes hardcoded):
  seq1, seq2: [1, 8192, 128] f32   node features
  adj:        [1, 8192, 8192] f32  dense adjacency
  cc_label:   [8, 1024] i32        community partition (arange layout)
  W: [128,128], b: [128], Wb: [128,128], bb: [] f32
  out:        [1, 16384] f32       = concat(ret1, ret2)

Math per GCN branch: h = relu(adj @ (seq @ W) + b), reassociated to
(adj @ seq) @ W so the big contraction uses seq tiles as the stationary
operand and a host-transposed adj block as the moving operand. Everything
lives in "transposed" space (features on partitions): the community mean
is a free-axis accumulation and the bilinear scores are 1-column matmuls.

Sharding: core k owns nodes [1024k, 1024k+1024) == community k (cc_label
is arange). Each core reads its adjT column block (16 MB fp16) + the full
seqs (4 MB, replicated). No collectives.

Schedule (the whole point of this version):
  - One merged seq tensor seq12[p, t, s, d] so seq DMAs interleave with
    adj groups on a single sync-queue FIFO in exact consumption order:
    seq[0:1], adj[0:1], seq[1:3], adj[1:2], seq[3:7], adj[2:4],
    seq[7:15], adj[4:8], then 7 groups of 8 adj tiles. The sync HWDGE
    queue alone sustains ~350-400 GB/s, and FIFO order == need order, so
    the PE starts ~1.2 us after the first bytes land and never starves.
  - Params + seq12[15:64] ride the scalar HWDGE queue (starts ~2.5 us
    later due to the act-table load; nothing early needs it).
  - All small matmuls (W-contraction, cw, scores) in fp16 (216 ns vs
    429 ns for fp32 at 512 cols). adj is pre-scaled by 256 on the host
    for fp16 range; the relu computes h' = relu(z + 256*b) and the 1/256
    is folded into host-side wbt (so scores come out right) and into the
    sigmoid scale (for the community mean).
  - m-dim split in two halves of 32 tiles; half-1's W-contraction +
    copies run mid-stream. For the last 16 m-tiles, branch 0 (which
    gates sigmoid -> cw -> all scores) streams before branch 1, so the
    whole branch-0 epilogue overlaps branch-1's matmuls and the
    post-stream tail is just branch-1's copy/W/relu/score chain.
  - Output via a single DMA on the sync queue (gpsimd never used; its
    software DGE drain cost ~1.8 us in the old version).
"""

import numpy as np

import concourse.bass as bass
import concourse.tile as tile
from concourse import bacc, mybir
from concourse.bass_utils import run_bass_kernel_spmd

N = 8192          # nodes
D = 128           # input feature dim
H = 128           # hidden dim
NC = 8            # communities / cores
CS = N // NC      # community size (nodes per core)
MT = N // 128     # number of 128-row m-tiles (64)
HALF = MT // 2
CHUNK = 512       # matmul moving free dim (psum bank width in fp32)
NCH = CS // CHUNK # n-chunks per core (2)

F32 = mybir.dt.float32
F16 = mybir.dt.float16
ADJ_SCALE = 256.0  # keeps fp16(adj*scale) in the normal range; undone via
                   # host-prescaled wbt (scores) and the sigmoid scale (mean)

# sync-queue interleave in FIFO == consumption order
SEQ_WARM = [(0, 1), (1, 3), (3, 7), (7, 15)]     # seq12 [start, end) tile ranges
ADJ_GROUPS = [(0, 1), (1, 1), (2, 2), (4, 4), (8, 8), (16, 8), (24, 8),
              (32, 8), (40, 8), (48, 8), (56, 8)]
SEQ_SCALAR = [(15, 12), (27, 12), (39, 12), (51, 13)]
TAIL_T0 = 48      # last 16 m-tiles stream branch 0 fully before branch 1


def _build_module() -> bass.Bass:
    nc = bacc.Bacc()

    adjt = nc.declare_dram_parameter("adjt", [128, MT, CS], F16, isOutput=False)
    seq = nc.declare_dram_parameter("seq", [128, MT, 2, D], F16, isOutput=False)
    w = nc.declare_dram_parameter("w", [D, H], F16, isOutput=False)
    wbt = nc.declare_dram_parameter("wbt", [H, H], F16, isOutput=False)
    bvec = nc.declare_dram_parameter("bvec", [H, 1], F32, isOutput=False)
    bbvec = nc.declare_dram_parameter("bbvec", [1, 1], F32, isOutput=False)
    out = nc.declare_dram_parameter("out", [2, CS], F32, isOutput=True)

    with tile.TileContext(nc) as tc:
        _emit(tc, adjt, seq, w, wbt, bvec, bbvec, out)
    nc.finalize()
    return nc


def _emit(tc, adjt, seq, w, wbt, bvec, bbvec, out):
    nc = tc.nc
    AF = mybir.ActivationFunctionType
    with (
        tc.tile_pool(name="singles", bufs=1) as singles,
        tc.tile_pool(name="adj_pool", bufs=1) as adj_pool,
        tc.tile_pool(name="misc", bufs=1) as misc,
        tc.tile_pool(name="psum", bufs=1, space="PSUM") as psum,
    ):
        # ---- DMA program. Sync queue first (it starts earliest), in
        # exact consumption order: seq warmup interleaved with adj groups.
        seq_sb = singles.tile([128, MT, 2, D], F16)
        adj_bufs = {}
        seq_iter = iter(SEQ_WARM)
        next_seq = next(seq_iter, None)
        for gi, (t0, gn) in enumerate(ADJ_GROUPS):
            while next_seq is not None and next_seq[0] < t0 + gn:
                s0, s1 = next_seq
                nc.sync.dma_start(out=seq_sb[:, s0:s1], in_=seq[:, s0:s1])
                next_seq = next(seq_iter, None)
            if gn < 8:
                buf = singles.tile([128, gn, CS], F16, name=f"adj_warm{gi}")
            else:
                buf = adj_pool.tile([128, gn, CS], F16, name="adj_sb",
                                    tag="adj_sb", bufs=4)
            nc.sync.dma_start(out=buf, in_=adjt[:, t0:t0 + gn, :])
            for u in range(gn):
                adj_bufs[t0 + u] = (buf, u)

        # Scalar queue: params then the bulk of seq12.
        w_sb = singles.tile([D, H], F16)
        nc.scalar.dma_start(out=w_sb, in_=w[:])
        wbt_sb = singles.tile([H, H], F16)
        nc.scalar.dma_start(out=wbt_sb, in_=wbt[:])
        b_sb = singles.tile([H, 1], F32)
        nc.scalar.dma_start(out=b_sb, in_=bvec[:])
        bb_sb = singles.tile([1, 1], F32)
        nc.scalar.dma_start(out=bb_sb, in_=bbvec[:])
        for s0, n in SEQ_SCALAR:
            nc.scalar.dma_start(out=seq_sb[:, s0:s0 + n], in_=seq[:, s0:s0 + n])

        # ---- Tiles.
        z = [
            [
                [psum.tile([128, CHUNK], F32, name=f"z_{h}_{s}_{c}") for c in range(NCH)]
                for s in range(2)
            ]
            for h in range(2)
        ]
        zt = [
            [
                [misc.tile([128, CHUNK], F16, name=f"zt_{h}_{s}_{c}") for c in range(NCH)]
                for s in range(2)
            ]
            for h in range(2)
        ]
        h_sb = [
            [misc.tile([128, CHUNK], F16, name=f"h_{s}_{c}") for c in range(NCH)]
            for s in range(2)
        ]
        csum = [misc.tile([H, 1], F32, name=f"csum_{c}") for c in range(NCH)]
        csum_tot = misc.tile([H, 1], F32)
        c_sb = misc.tile([H, 1], F16)
        cw_sb = misc.tile([H, 1], F16)
        out_sb = misc.tile([1, 2, CS], F32)

        def mm(t, s):
            lhsT = seq_sb[:, t, s, :]
            for c in range(NCH):
                buf, u = adj_bufs[t]
                nc.tensor.matmul(
                    z[t // HALF][s][c],
                    lhsT,
                    buf[:, u, c * CHUNK:(c + 1) * CHUNK],
                    start=(t % HALF == 0),
                    stop=(t % HALF == HALF - 1),
                )

        def copy_z(h, s):
            # psum fp32 -> sbuf fp16, chunk 0 on vector / chunk 1 on scalar
            nc.vector.tensor_copy(out=zt[h][s][0], in_=z[h][s][0])
            nc.scalar.activation(out=zt[h][s][1], in_=z[h][s][1], func=AF.Copy)

        def wagg(h, s, start, stop):
            for c in range(NCH):
                nc.tensor.matmul(z[0][s][c], w_sb, zt[h][s][c], start=start, stop=stop)

        # ---- Main stream.
        for t in range(TAIL_T0):
            for s in range(2):
                mm(t, s)
            if t == HALF - 1:
                for s in range(2):
                    copy_z(0, s)
            if t == HALF + 15:
                for s in range(2):
                    wagg(0, s, start=True, stop=False)

        # Last 16 tiles: branch 0 fully first, then its epilogue (which
        # overlaps branch 1's matmuls), then branch 1 + its epilogue.
        for t in range(TAIL_T0, MT):
            mm(t, 0)

        copy_z(1, 0)
        wagg(1, 0, start=False, stop=True)
        for c in range(NCH):
            nc.scalar.activation(
                out=h_sb[0][c],
                in_=z[0][0][c],
                func=AF.Relu,
                bias=b_sb,
                accum_out=csum[c],
            )
        nc.vector.tensor_add(out=csum_tot, in0=csum[0], in1=csum[1])
        nc.scalar.activation(
            out=c_sb, in_=csum_tot, func=AF.Sigmoid, scale=1.0 / (CS * ADJ_SCALE)
        )
        cw_ps = z[1][0][0]
        nc.tensor.matmul(cw_ps[:, :1], wbt_sb, c_sb, start=True, stop=True)
        nc.vector.tensor_copy(out=cw_sb, in_=cw_ps[:, :1])
        # branch-0 scores into banks freed by the branch-0 copies/relu
        sc0 = [z[1][0][1], z[0][0][0]]
        for c in range(NCH):
            nc.tensor.matmul(sc0[c][:1, :], cw_sb, h_sb[0][c], start=True, stop=True)
        nc.vector.tensor_scalar_add(
            out=out_sb[:, 0, 0:CHUNK], in0=sc0[0][:1, :], scalar1=bb_sb
        )
        nc.scalar.activation(
            out=out_sb[:, 0, CHUNK:], in_=sc0[1][:1, :], func=AF.Identity, bias=bb_sb
        )

        for t in range(TAIL_T0, MT):
            mm(t, 1)

        copy_z(1, 1)
        wagg(1, 1, start=False, stop=True)
        nc.vector.tensor_scalar(
            out=h_sb[1][0],
            in0=z[0][1][0],
            scalar1=b_sb,
            scalar2=0.0,
            op0=mybir.AluOpType.add,
            op1=mybir.AluOpType.max,
        )
        nc.scalar.activation(
            out=h_sb[1][1], in_=z[0][1][1], func=AF.Relu, bias=b_sb
        )
        sc1 = [z[1][1][0], z[1][1][1]]
        for c in range(NCH):
            nc.tensor.matmul(sc1[c][:1, :], cw_sb, h_sb[1][c], start=True, stop=True)
        nc.vector.tensor_scalar_add(
            out=out_sb[:, 1, 0:CHUNK], in0=sc1[0][:1, :], scalar1=bb_sb
        )
        nc.scalar.activation(
            out=out_sb[:, 1, CHUNK:], in_=sc1[1][:1, :], func=AF.Identity, bias=bb_sb
        )
        nc.sync.dma_start(out=out[:, :].unsqueeze(0), in_=out_sb)


_MODULE_CACHE: list = []


def get_module() -> bass.Bass:
    if not _MODULE_CACHE:
        _MODULE_CACHE.append(_build_module())
    return _MODULE_CACHE[0]


def shard_inputs(inputs: dict) -> list[dict]:
    """Full inputs -> per-core input maps (row-block sharding of adjT)."""
    s1 = np.asarray(inputs["seq1"], np.float32)[0].astype(np.float16)
    s2 = np.asarray(inputs["seq2"], np.float32)[0].astype(np.float16)
    # seq12[p, t, s, d] = seq_s[128*t + p, d]
    seq12 = np.ascontiguousarray(
        np.stack([s1, s2], axis=0).reshape(2, MT, 128, D).transpose(2, 1, 0, 3)
    )
    adj16 = (np.asarray(inputs["adj"], np.float32)[0] * ADJ_SCALE).astype(np.float16)
    w = np.asarray(inputs["W"], np.float32).astype(np.float16)
    wbt = (np.asarray(inputs["Wb"], np.float32).T / ADJ_SCALE).astype(np.float16)
    wbt = np.ascontiguousarray(wbt)
    bvec = (np.asarray(inputs["b"], np.float32) * ADJ_SCALE).reshape(H, 1).copy()
    bbvec = np.asarray(inputs["bb"], np.float32).reshape(1, 1).copy()

    in_maps = []
    for k in range(NC):
        in_maps.append(
            {
                "adjt": np.ascontiguousarray(
                    adj16[k * CS:(k + 1) * CS, :].T.reshape(MT, 128, CS).transpose(1, 0, 2)
                ),
                "seq": seq12,
                "w": w,
                "wbt": wbt,
                "bvec": bvec,
                "bbvec": bbvec,
            }
        )
    return in_maps


def gather_output(core_outs: list[np.ndarray], cc_label: np.ndarray) -> np.ndarray:
    """Per-core [2, CS] score blocks -> full [1, 2N] output.

    Scatter through cc_label mirrors the reference's .at[flat].set: entry
    (community k, position j) is the score of node cc_label[k, j].
    """
    sc1 = np.concatenate([o[0] for o in core_outs]).astype(np.float32)
    sc2 = np.concatenate([o[1] for o in core_outs]).astype(np.float32)
    flat = np.asarray(cc_label).reshape(-1)
    ret1 = np.zeros(N, np.float32)
    ret2 = np.zeros(N, np.float32)
    ret1[flat] = sc1
    ret2[flat] = sc2
    return np.concatenate([ret1, ret2])[None, :]


def kernel(**inputs) -> np.ndarray:
    nc = get_module()
    in_maps = shard_inputs(inputs)
    res = run_bass_kernel_spmd(nc, in_maps, core_ids=list(range(NC)))
    core_outs = [res.results[k]["out"] for k in range(NC)]
    return gather_output(core_outs, inputs["cc_label"])


if __name__ == "__main__":
    nc = get_module()
    print("module built ok")


# revision 11
# speedup vs baseline: 1.0619x; 1.0331x over previous
"""DGI (Deep Graph Infomax) forward kernel for 8 TRN2 NeuronCores.

Problem (all shapes hardcoded):
  seq1, seq2: [1, 8192, 128] f32   node features
  adj:        [1, 8192, 8192] f32  dense adjacency
  cc_label:   [8, 1024] i32        community partition (arange layout)
  W: [128,128], b: [128], Wb: [128,128], bb: [] f32
  out:        [1, 16384] f32       = concat(ret1, ret2)

Math per GCN branch: h = relu(adj @ (seq @ W) + b), reassociated to
(adj @ seq) @ W so the big contraction uses seq tiles as the stationary
operand and a host-transposed adj block as the moving operand. Everything
lives in "transposed" space (features on partitions): the community mean
is a free-axis accumulation and the bilinear scores are 1-column matmuls.

Sharding: core k owns nodes [1024k, 1024k+1024) == community k (cc_label
is arange). Each core reads its adjT column block (16 MB fp16) + the full
seqs (4 MB, replicated). No collectives.

Schedule (the whole point of this version):
  - One merged seq tensor seq12[p, t, s, d] so seq DMAs interleave with
    adj groups on a single sync-queue FIFO in exact consumption order:
    seq[0:1], adj[0:1], seq[1:3], adj[1:2], seq[3:7], adj[2:4],
    seq[7:15], adj[4:8], then 7 groups of 8 adj tiles. The sync HWDGE
    queue alone sustains ~350-400 GB/s, and FIFO order == need order, so
    the PE starts ~1.2 us after the first bytes land and never starves.
  - Params + seq12[15:64] ride the scalar HWDGE queue (starts ~2.5 us
    later due to the act-table load; nothing early needs it).
  - All small matmuls (W-contraction, cw, scores) in fp16 (216 ns vs
    429 ns for fp32 at 512 cols). adj is pre-scaled by 256 on the host
    for fp16 range; the relu computes h' = relu(z + 256*b) and the 1/256
    is folded into host-side wbt (so scores come out right) and into the
    sigmoid scale (for the community mean).
  - m-dim split in two halves of 32 tiles; half-1's W-contraction +
    copies run mid-stream. For the last 16 m-tiles, branch 0 (which
    gates sigmoid -> cw -> all scores) streams before branch 1, so the
    whole branch-0 epilogue overlaps branch-1's matmuls and the
    post-stream tail is just branch-1's copy/W/relu/score chain.
  - Output via a single DMA on the sync queue (gpsimd never used; its
    software DGE drain cost ~1.8 us in the old version).
"""

import numpy as np

import concourse.bass as bass
import concourse.tile as tile
from concourse import bacc, mybir
from concourse.bass_utils import run_bass_kernel_spmd

N = 8192          # nodes
D = 128           # input feature dim
H = 128           # hidden dim
NC = 8            # communities / cores
CS = N // NC      # community size (nodes per core)
MT = N // 128     # number of 128-row m-tiles (64)
HALF = MT // 2
CHUNK = 512       # matmul moving free dim (psum bank width in fp32)
NCH = CS // CHUNK # n-chunks per core (2)

F32 = mybir.dt.float32
F16 = mybir.dt.float16
ADJ_SCALE = 256.0  # keeps fp16(adj*scale) in the normal range; undone via
                   # host-prescaled wbt (scores) and the sigmoid scale (mean)

# sync-queue interleave in FIFO == consumption order. The DMA path ramps
# for its first ~10 us (~150-250 GB/s before settling at ~390-430), so the
# early adjacency is fetched at fine granularity (per-chunk, then per-tile,
# then per-2-tile) into one big warm tile -- byte-granular dependency
# tracking lets each matmul start as soon as ITS bytes land instead of
# waiting for a 2 MB group.
SEQ_WARM = [(0, 1), (1, 3), (3, 7), (7, 15)]     # seq12 [start, end) tile ranges
WARM_NT = 24                                      # adj tiles fetched fine-grained
ADJ_GROUPS = [(24, 8), (32, 8), (40, 8), (48, 8), (56, 8)]
SEQ_SCALAR = [(15, 12), (27, 12), (39, 12), (51, 13)]
TAIL_T0 = 48      # last 16 m-tiles stream branch 0 fully before branch 1


def _build_module() -> bass.Bass:
    nc = bacc.Bacc()

    adjt = nc.declare_dram_parameter("adjt", [128, MT, CS], F16, isOutput=False)
    seq = nc.declare_dram_parameter("seq", [128, MT, 2, D], F16, isOutput=False)
    w = nc.declare_dram_parameter("w", [D, H], F16, isOutput=False)
    wbt = nc.declare_dram_parameter("wbt", [H, H], F16, isOutput=False)
    bvec = nc.declare_dram_parameter("bvec", [H, 1], F32, isOutput=False)
    bbvec = nc.declare_dram_parameter("bbvec", [1, 1], F32, isOutput=False)
    out = nc.declare_dram_parameter("out", [2, CS], F32, isOutput=True)

    with tile.TileContext(nc) as tc:
        _emit(tc, adjt, seq, w, wbt, bvec, bbvec, out)
    nc.finalize()
    return nc


def _emit(tc, adjt, seq, w, wbt, bvec, bbvec, out):
    nc = tc.nc
    AF = mybir.ActivationFunctionType
    with (
        tc.tile_pool(name="singles", bufs=1) as singles,
        tc.tile_pool(name="adj_pool", bufs=1) as adj_pool,
        tc.tile_pool(name="misc", bufs=1) as misc,
        tc.tile_pool(name="psum", bufs=1, space="PSUM") as psum,
    ):
        # ---- DMA program. Sync queue first (it starts earliest), in
        # exact consumption order: seq warmup interleaved with the
        # fine-grained adj warm region, then 8-tile groups.
        seq_sb = singles.tile([128, MT, 2, D], F16)
        warm_sb = singles.tile([128, WARM_NT, CS], F16)
        adj_bufs = {t: (warm_sb, t) for t in range(WARM_NT)}

        seq_iter = iter(SEQ_WARM)
        next_seq = next(seq_iter, None)

        def seq_upto(t):
            nonlocal next_seq
            while next_seq is not None and next_seq[0] <= t:
                s0, s1 = next_seq
                nc.sync.dma_start(out=seq_sb[:, s0:s1], in_=seq[:, s0:s1])
                next_seq = next(seq_iter, None)

        # per-chunk for t 0-1, per-tile to 11, per-2-tile to 23
        for t in range(2):
            seq_upto(t)
            for c in range(NCH):
                nc.sync.dma_start(
                    out=warm_sb[:, t, c * CHUNK:(c + 1) * CHUNK],
                    in_=adjt[:, t, c * CHUNK:(c + 1) * CHUNK],
                )
        for t in range(2, 12):
            seq_upto(t)
            nc.sync.dma_start(out=warm_sb[:, t, :], in_=adjt[:, t, :])
        for t in range(12, WARM_NT, 2):
            seq_upto(t + 1)
            nc.sync.dma_start(out=warm_sb[:, t:t + 2, :], in_=adjt[:, t:t + 2, :])
        for t0, gn in ADJ_GROUPS:
            buf = adj_pool.tile([128, gn, CS], F16, name="adj_sb",
                                tag="adj_sb", bufs=3)
            nc.sync.dma_start(out=buf, in_=adjt[:, t0:t0 + gn, :])
            for u in range(gn):
                adj_bufs[t0 + u] = (buf, u)

        # Scalar queue: params then the bulk of seq12.
        w_sb = singles.tile([D, H], F16)
        nc.scalar.dma_start(out=w_sb, in_=w[:])
        wbt_sb = singles.tile([H, H], F16)
        nc.scalar.dma_start(out=wbt_sb, in_=wbt[:])
        b_sb = singles.tile([H, 1], F32)
        nc.scalar.dma_start(out=b_sb, in_=bvec[:])
        bb_sb = singles.tile([1, 1], F32)
        nc.scalar.dma_start(out=bb_sb, in_=bbvec[:])
        for s0, n in SEQ_SCALAR:
            nc.scalar.dma_start(out=seq_sb[:, s0:s0 + n], in_=seq[:, s0:s0 + n])

        # ---- Tiles.
        z = [
            [
                [psum.tile([128, CHUNK], F32, name=f"z_{h}_{s}_{c}") for c in range(NCH)]
                for s in range(2)
            ]
            for h in range(2)
        ]
        zt = [
            [
                [misc.tile([128, CHUNK], F16, name=f"zt_{h}_{s}_{c}") for c in range(NCH)]
                for s in range(2)
            ]
            for h in range(2)
        ]
        h_sb = [
            [misc.tile([128, CHUNK], F16, name=f"h_{s}_{c}") for c in range(NCH)]
            for s in range(2)
        ]
        csum = [misc.tile([H, 1], F32, name=f"csum_{c}") for c in range(NCH)]
        csum_tot = misc.tile([H, 1], F32)
        c_sb = misc.tile([H, 1], F16)
        cw_sb = misc.tile([H, 1], F16)
        out_sb = misc.tile([1, 2, CS], F32)

        def mm(t, s, cs=(0, 1)):
            lhsT = seq_sb[:, t, s, :]
            for c in cs:
                buf, u = adj_bufs[t]
                nc.tensor.matmul(
                    z[t // HALF][s][c],
                    lhsT,
                    buf[:, u, c * CHUNK:(c + 1) * CHUNK],
                    start=(t % HALF == 0),
                    stop=(t % HALF == HALF - 1),
                )

        def copy_z(h, s):
            # psum fp32 -> sbuf fp16, chunk 0 on vector / chunk 1 on scalar
            nc.vector.tensor_copy(out=zt[h][s][0], in_=z[h][s][0])
            nc.scalar.activation(out=zt[h][s][1], in_=z[h][s][1], func=AF.Copy)

        def wagg(h, s, start, stop):
            for c in range(NCH):
                nc.tensor.matmul(z[0][s][c], w_sb, zt[h][s][c], start=start, stop=stop)

        # ---- Main stream.
        for t in range(TAIL_T0):
            for s in range(2):
                mm(t, s)
            if t == HALF - 1:
                for s in range(2):
                    copy_z(0, s)
            if t == HALF + 15:
                for s in range(2):
                    wagg(0, s, start=True, stop=False)

        # Last 16 tiles: branch 0 streams fully first; its epilogue's PE ops
        # are then interleaved between branch-1 matmul batches (the PE queue
        # is in-order, so emission position == queue position) to hide the
        # scalar-side relu/sigmoid chain. Branch 1 finishes c-major over the
        # last 3 tiles so chunk 0's epilogue overlaps chunk 1's matmuls.
        for t in range(TAIL_T0, MT):
            mm(t, 0)
        copy_z(1, 0)
        for t in range(TAIL_T0, TAIL_T0 + 4):
            mm(t, 1)
        wagg(1, 0, start=False, stop=True)
        for c in range(NCH):
            nc.scalar.activation(
                out=h_sb[0][c],
                in_=z[0][0][c],
                func=AF.Relu,
                bias=b_sb,
                accum_out=csum[c],
            )
        nc.vector.tensor_add(out=csum_tot, in0=csum[0], in1=csum[1])
        nc.scalar.activation(
            out=c_sb, in_=csum_tot, func=AF.Sigmoid, scale=1.0 / (CS * ADJ_SCALE)
        )
        for t in range(TAIL_T0 + 4, TAIL_T0 + 9):
            mm(t, 1)
        cw_ps = z[1][0][0]
        nc.tensor.matmul(cw_ps[:, :1], wbt_sb, c_sb, start=True, stop=True)
        nc.vector.tensor_copy(out=cw_sb, in_=cw_ps[:, :1])
        for t in range(TAIL_T0 + 9, TAIL_T0 + 11):
            mm(t, 1)
        # branch-0 scores into banks freed by the branch-0 copies/relu
        sc0 = [z[1][0][1], z[0][0][0]]
        for c in range(NCH):
            nc.tensor.matmul(sc0[c][:1, :], cw_sb, h_sb[0][c], start=True, stop=True)
        nc.vector.tensor_scalar_add(
            out=out_sb[:, 0, 0:CHUNK], in0=sc0[0][:1, :], scalar1=bb_sb
        )
        nc.scalar.activation(
            out=out_sb[:, 0, CHUNK:], in_=sc0[1][:1, :], func=AF.Identity, bias=bb_sb
        )
        for t in range(TAIL_T0 + 11, MT - 3):
            mm(t, 1)
        for t in range(MT - 3, MT):
            mm(t, 1, cs=(0,))
        nc.vector.tensor_copy(out=zt[1][1][0], in_=z[1][1][0])
        for t in range(MT - 3, MT):
            mm(t, 1, cs=(1,))
        nc.tensor.matmul(z[0][1][0], w_sb, zt[1][1][0], start=False, stop=True)
        nc.scalar.activation(out=zt[1][1][1], in_=z[1][1][1], func=AF.Copy)
        nc.vector.tensor_scalar(
            out=h_sb[1][0],
            in0=z[0][1][0],
            scalar1=b_sb,
            scalar2=0.0,
            op0=mybir.AluOpType.add,
            op1=mybir.AluOpType.max,
        )
        nc.tensor.matmul(z[0][1][1], w_sb, zt[1][1][1], start=False, stop=True)
        sc1 = [z[1][1][0], z[1][1][1]]
        nc.tensor.matmul(sc1[0][:1, :], cw_sb, h_sb[1][0], start=True, stop=True)
        nc.scalar.activation(
            out=h_sb[1][1], in_=z[0][1][1], func=AF.Relu, bias=b_sb
        )
        nc.vector.tensor_scalar_add(
            out=out_sb[:, 1, 0:CHUNK], in0=sc1[0][:1, :], scalar1=bb_sb
        )
        nc.tensor.matmul(sc1[1][:1, :], cw_sb, h_sb[1][1], start=True, stop=True)
        nc.scalar.activation(
            out=out_sb[:, 1, CHUNK:], in_=sc1[1][:1, :], func=AF.Identity, bias=bb_sb
        )
        nc.sync.dma_start(out=out[:, :].unsqueeze(0), in_=out_sb)


_MODULE_CACHE: list = []


def get_module() -> bass.Bass:
    if not _MODULE_CACHE:
        _MODULE_CACHE.append(_build_module())
    return _MODULE_CACHE[0]


def shard_inputs(inputs: dict) -> list[dict]:
    """Full inputs -> per-core input maps (row-block sharding of adjT)."""
    s1 = np.asarray(inputs["seq1"], np.float32)[0].astype(np.float16)
    s2 = np.asarray(inputs["seq2"], np.float32)[0].astype(np.float16)
    # seq12[p, t, s, d] = seq_s[128*t + p, d]
    seq12 = np.ascontiguousarray(
        np.stack([s1, s2], axis=0).reshape(2, MT, 128, D).transpose(2, 1, 0, 3)
    )
    adj16 = (np.asarray(inputs["adj"], np.float32)[0] * ADJ_SCALE).astype(np.float16)
    w = np.asarray(inputs["W"], np.float32).astype(np.float16)
    wbt = (np.asarray(inputs["Wb"], np.float32).T / ADJ_SCALE).astype(np.float16)
    wbt = np.ascontiguousarray(wbt)
    bvec = (np.asarray(inputs["b"], np.float32) * ADJ_SCALE).reshape(H, 1).copy()
    bbvec = np.asarray(inputs["bb"], np.float32).reshape(1, 1).copy()

    in_maps = []
    for k in range(NC):
        in_maps.append(
            {
                "adjt": np.ascontiguousarray(
                    adj16[k * CS:(k + 1) * CS, :].T.reshape(MT, 128, CS).transpose(1, 0, 2)
                ),
                "seq": seq12,
                "w": w,
                "wbt": wbt,
                "bvec": bvec,
                "bbvec": bbvec,
            }
        )
    return in_maps


def gather_output(core_outs: list[np.ndarray], cc_label: np.ndarray) -> np.ndarray:
    """Per-core [2, CS] score blocks -> full [1, 2N] output.

    Scatter through cc_label mirrors the reference's .at[flat].set: entry
    (community k, position j) is the score of node cc_label[k, j].
    """
    sc1 = np.concatenate([o[0] for o in core_outs]).astype(np.float32)
    sc2 = np.concatenate([o[1] for o in core_outs]).astype(np.float32)
    flat = np.asarray(cc_label).reshape(-1)
    ret1 = np.zeros(N, np.float32)
    ret2 = np.zeros(N, np.float32)
    ret1[flat] = sc1
    ret2[flat] = sc2
    return np.concatenate([ret1, ret2])[None, :]


def kernel(**inputs) -> np.ndarray:
    nc = get_module()
    in_maps = shard_inputs(inputs)
    res = run_bass_kernel_spmd(nc, in_maps, core_ids=list(range(NC)))
    core_outs = [res.results[k]["out"] for k in range(NC)]
    return gather_output(core_outs, inputs["cc_label"])


if __name__ == "__main__":
    nc = get_module()
    print("module built ok")
